# revision 1
# baseline (speedup 1.0000x reference)
"""Trainium2 Bass kernel for nn_Decoder_36953898615460.

recon[B, D] = einsum('lbf,lfd->bd', acts[:n], W[:n]) + sum(bias[:n], 0)

Strategy (row-parallel over F, 8 NeuronCores):
  - Shard the contraction dim F across 8 cores: core r owns F columns
    [r*768, (r+1)*768)  ->  local contraction K_loc = n*768.
  - Host prep (pure layout): acts shard transposed to [K_loc, B] so the
    contraction dim lands on SBUF partitions with contiguous DMA; W shard
    reshaped to [K_loc, D]; bias transposed to [D, n].
  - Per core: partial[D, B] (output transposed: d on partitions) computed
    as fp32r (TF32) matmuls accumulating in PSUM per K-chunk, chunk results
    accumulated into an SBUF fp32 accumulator.
  - B is processed in two halves, each with a full K pass and its own
    ReduceScatter(add); the first RS overlaps the second half's compute,
    so only the second RS is exposed at the tail. (W is streamed twice;
    DMA stays under the PE roofline.)
  - bias: each core adds sum_l(bias)/8 so the 8-way reduce sums to +bias.
  - Device-side ReduceScatter(add): core r ends with rows [r*96, (r+1)*96)
    of the reduced [D, B].
  - Host: concat the 8 shards -> [D, B], transpose -> [B, D].
"""

import numpy as np

import concourse.mybir as mybir
import concourse.tile as tile
from concourse import bacc
from concourse.bass import ts
from concourse.bass_utils import run_bass_kernel_spmd

NCORES = 8
B, F, D = 2048, 6144, 768
F_LOC = F // NCORES  # 768
P = 128
NFREE = 512          # matmul moving free dim (one PSUM bank of fp32)
CK = 6               # k-tiles (of 128) per chunk
HALVES = 2           # B split; each half gets a full K pass + its own RS
# Symmetric split: half-1's 227 us PE window absorbs the W re-stream plus
# RS_0's SDMA traffic with margin (a 1536/512 split starves half-1's DMA:
# measured 618 us vs 588 us symmetric).
BHS = [1024, 1024]
BOFF = [0, 1024]
PARTIAL_DT = mybir.dt.float32  # RS wire format (bf16 would be ~4% faster at ~20x the error; fp32 keeps rel err at the fp32r compute floor 1.5e-4)

_nc_cache = {}
last_result = None  # BassKernelResults of the most recent run (for test harness)


def _build(n_layers: int):
    K_LOC = n_layers * F_LOC          # 9216 for n=12
    KT = K_LOC // P                   # 72 k-tiles
    assert KT % CK == 0
    NCH = KT // CK                    # 12 chunks
    MD = D // P                       # 6 d-subtiles
    DR = D // NCORES                  # 96 rows per rank after ReduceScatter

    nc = bacc.Bacc(None, num_devices=NCORES)
    a_ext = nc.dram_tensor("a_t", [K_LOC, B], mybir.dt.float32r, kind="ExternalInput")
    w_ext = nc.dram_tensor("w", [K_LOC, D], mybir.dt.float32r, kind="ExternalInput")
    b_ext = nc.dram_tensor("bias_t", [D, n_layers], mybir.dt.float32, kind="ExternalInput")
    y_ext = nc.dram_tensor("y", [DR, B], PARTIAL_DT, kind="ExternalOutput")

    partials = [
        nc.dram_tensor(f"partial{h}", [D, BHS[h]], PARTIAL_DT) for h in range(HALVES)
    ]
    reduceds = [
        nc.dram_tensor(f"reduced{h}", [DR, BHS[h]], PARTIAL_DT) for h in range(HALVES)
    ]

    a_v = a_ext[:, :].rearrange("(ko p) b -> p ko b", p=P)  # [128, KT, B]
    w_v = w_ext[:, :].rearrange("(ko p) d -> p ko d", p=P)  # [128, KT, D]
    b_v = b_ext[:, :].rearrange("(mo p) l -> p mo l", p=P)  # [128, MD, n]

    with tile.TileContext(nc) as tc:
        with (
            tc.tile_pool(name="apool", bufs=2) as apool,
            tc.tile_pool(name="wpool", bufs=2) as wpool,
            tc.tile_pool(name="cpool", bufs=1) as cpool,
            tc.tile_pool(name="opool", bufs=2) as opool,
            tc.tile_pool(name="pspool", bufs=3, space="PSUM") as pspool,
        ):
            # bias8[p, mo] = sum_l bias[l, mo*128+p] / NCORES
            bias_t = cpool.tile([P, MD, n_layers], mybir.dt.float32)
            nc.sync.dma_start(bias_t[:], b_v)
            bias8 = cpool.tile([P, MD], mybir.dt.float32)
            nc.vector.reduce_sum(bias8[:], bias_t[:], axis=mybir.AxisListType.X)
            nc.vector.tensor_scalar_mul(bias8[:], bias8[:], 1.0 / NCORES)

            for h in range(HALVES):
                b0, BH = BOFF[h], BHS[h]
                NB = BH // NFREE
                # fp32 accumulator for this half's partial, acc[p, mo, b]
                acc = cpool.tile([P, MD, BH], mybir.dt.float32, tag="acc")
                for c in range(NCH):
                    a_c = apool.tile([P, CK, BH], mybir.dt.float32r, tag="a")
                    w_c = wpool.tile([P, CK, D], mybir.dt.float32r, tag="w")
                    for k in range(CK):
                        nc.sync.dma_start(
                            a_c[:, k], a_v[:, c * CK + k, b0 : b0 + BH]
                        )
                        nc.sync.dma_start(w_c[:, k], w_v[:, c * CK + k])
                    for m in range(MD):
                        ps = pspool.tile([P, BH], mybir.dt.float32, tag="ps")
                        for k in range(CK):
                            lhsT = w_c[:, k, ts(m, P)]
                            for nb in range(NB):
                                nc.tensor.matmul(
                                    ps[:, ts(nb, NFREE)],
                                    lhsT,
                                    a_c[:, k, ts(nb, NFREE)],
                                    start=(k == 0),
                                    stop=(k == CK - 1),
                                )
                        if c == 0:
                            nc.vector.tensor_scalar_add(
                                acc[:, m], ps[:], bias8[:, m : m + 1]
                            )
                        else:
                            nc.vector.tensor_add(acc[:, m], ps[:], acc[:, m])

                # write this half's partial (convert only if wire dtype differs)
                for m in range(MD):
                    if PARTIAL_DT == mybir.dt.float32:
                        nc.sync.dma_start(partials[h][ts(m, P), :], acc[:, m])
                    else:
                        pb = opool.tile([P, BH], PARTIAL_DT, tag="pb")
                        nc.vector.tensor_copy(pb[:], acc[:, m])
                        nc.sync.dma_start(partials[h][ts(m, P), :], pb[:])

                nc.gpsimd.collective_compute(
                    "ReduceScatter",
                    mybir.AluOpType.add,
                    replica_groups=[list(range(NCORES))],
                    ins=[partials[h][:, :].opt()],
                    outs=[reduceds[h][:, :].opt()],
                )

            # Final output DMAs last, on the SWDGE (gpsimd) queue: a y-DMA
            # waits on its RS completion, and a waiting DMA at the head of
            # the sync HWDGE queue would stall the second half's input
            # streaming behind it (measured 41 us PE gap).
            for h in range(HALVES):
                nc.gpsimd.dma_start(
                    y_ext[:, BOFF[h] : BOFF[h] + BHS[h]], reduceds[h][:, :]
                )
    nc.compile()
    return nc


def _get_nc(n_layers: int):
    if n_layers not in _nc_cache:
        _nc_cache[n_layers] = _build(n_layers)
    return _nc_cache[n_layers]


def kernel(acts: np.ndarray, W: np.ndarray, bias: np.ndarray, layer_idx) -> np.ndarray:
    global last_result
    n = int(layer_idx) + 1
    acts = np.asarray(acts, dtype=np.float32)[:n]  # [n, B, F]
    W = np.asarray(W, dtype=np.float32)[:n]        # [n, F, D]
    bias = np.asarray(bias, dtype=np.float32)[:n]  # [n, D]

    nc = _get_nc(n)

    bias_t = np.ascontiguousarray(bias.T)  # [D, n], same on every core
    in_maps = []
    for r in range(NCORES):
        f0 = r * F_LOC
        # [n, B, F_LOC] -> [n, F_LOC, B] -> [K_loc, B]
        a_t = np.ascontiguousarray(acts[:, :, f0 : f0 + F_LOC].transpose(0, 2, 1)).reshape(
            n * F_LOC, B
        )
        w_r = np.ascontiguousarray(W[:, f0 : f0 + F_LOC, :]).reshape(n * F_LOC, D)
        in_maps.append({"a_t": a_t, "w": w_r, "bias_t": bias_t})

    last_result = run_bass_kernel_spmd(nc, in_maps, core_ids=list(range(NCORES)))
    out_t = np.concatenate([last_result.results[r]["y"] for r in range(NCORES)], axis=0)
    return np.ascontiguousarray(out_t.T.astype(np.float32))  # [B, D] float32



# revision 2
# speedup vs baseline: 1.0426x; 1.0426x over previous
"""Trainium2 Bass kernel for nn_Decoder_36953898615460 (v2: bf16, PE-bound).

recon[B, D] = einsum('lbf,lfd->bd', acts[:n], W[:n]) + sum(bias[:n], 0)

Strategy (row-parallel over F, 8 NeuronCores):
  - Shard the contraction dim F across 8 cores: core r owns F columns
    [r*768, (r+1)*768)  ->  local contraction K_loc = n*768 (9216 for n=12).
  - Host prep: inputs cast to bf16 (rel err ~2.4e-3 << 2e-2 tol); acts shard
    transposed to [K_loc, B]; W shard reshaped to [K_loc, D]; bias -> [D, n].
  - bf16 halves HBM traffic vs fp32/fp32r: per-core DMA-in ~52 MB (145 us)
    vs the PE floor 29 GF / 78.6 TF/s = 369 us -> cleanly PE-bound.
  - W (14.2 MB bf16) stays SBUF-resident: streamed once during block 0 on the
    ACT HWDGE queue (parallel with acts streaming on the SP queue).
  - B processed in 4 blocks of 512. Per block, the full K accumulation for
    each of the 6 d-subtiles happens in a single PSUM bank (72 accumulating
    matmuls, start/stop flags) -- no SBUF fp32 accumulator, no per-chunk
    vector adds (the v1 kernel burned ~157 us of DVE on those).
  - Per-block ReduceScatter(add) overlaps the next block's compute; only the
    last block's RS is exposed at the tail.
  - Each block's LAST k-chunk runs m-outer with the evacuation issued right
    after that subtile's stop-matmul, so 5 of 6 evacuations + partial DMAs
    hide under the remaining matmuls and the RS starts ~2 us after the last
    matmul instead of ~8 us.
  - bias: each core adds sum_l(bias)/8 during PSUM->SBUF evacuation so the
    8-way reduce sums to +bias. Evacuations alternate DVE/ACT engines.
  - Queues: acts + W streaming interleaved on the SP (sync) HWDGE queue;
    partial writes on the ACT (scalar) HWDGE queue so they never
    head-of-line-block the input stream; bias on SWDGE; RS + final y DMAs on
    the gpsimd SWDGE queue.
  - Host: concat the 8 [96, 2048] shards -> [768, 2048], transpose -> [B, D].
"""

import numpy as np
import ml_dtypes

import concourse.mybir as mybir
import concourse.tile as tile
from concourse import bacc
from concourse.bass import ts
from concourse.bass_utils import run_bass_kernel_spmd

NCORES = 8
B, F, D = 2048, 6144, 768
F_LOC = F // NCORES  # 768
P = 128
MD = D // P          # 6 d-subtiles
DR = D // NCORES     # 96 rows per rank after ReduceScatter
BN = 512             # B block width (= matmul moving free dim, one PSUM bank)
NBLK = B // BN       # 4
CK = 8               # preferred k-tiles (of 128) per DMA chunk

_nc_cache = {}
last_result = None  # BassKernelResults of the most recent run (for test harness)


def _build(n_layers: int):
    K_LOC = n_layers * F_LOC          # 9216 for n=12
    KT = K_LOC // P                   # 72 k-tiles
    ck = max(c for c in (CK, 6, 4, 3, 2, 1) if KT % c == 0)
    NCH = KT // ck                    # 9 chunks for n=12

    nc = bacc.Bacc(None, num_devices=NCORES)
    a_ext = nc.dram_tensor("a_t", [K_LOC, B], mybir.dt.bfloat16, kind="ExternalInput")
    w_ext = nc.dram_tensor("w", [K_LOC, D], mybir.dt.bfloat16, kind="ExternalInput")
    b_ext = nc.dram_tensor("bias_t", [D, n_layers], mybir.dt.float32, kind="ExternalInput")
    # block-major output so each block's reduced shard lands contiguously
    y_ext = nc.dram_tensor("y", [NBLK, DR, BN], mybir.dt.float32, kind="ExternalOutput")

    # bf16 wire format for the reduce: halves RS payload + partial DMAs.
    # Adds ~1e-3 quantization on partials (total rel err stays ~2.6e-3).
    partials = [nc.dram_tensor(f"partial{b}", [D, BN], mybir.dt.bfloat16) for b in range(NBLK)]
    reduceds = [nc.dram_tensor(f"reduced{b}", [DR, BN], mybir.dt.bfloat16) for b in range(NBLK)]

    a_v = a_ext[:, :].rearrange("(ko p) b -> p ko b", p=P)  # [128, KT, B]
    w_v = w_ext[:, :].rearrange("(ko p) d -> p ko d", p=P)  # [128, KT, D]
    b_v = b_ext[:, :].rearrange("(mo p) l -> p mo l", p=P)  # [128, MD, n]

    with tile.TileContext(nc) as tc:
        with (
            tc.tile_pool(name="apool", bufs=3) as apool,
            tc.tile_pool(name="wpool", bufs=NCH) as wpool,
            tc.tile_pool(name="cpool", bufs=1) as cpool,
            tc.tile_pool(name="opool", bufs=3) as opool,
            tc.tile_pool(name="pspool", bufs=8, space="PSUM") as pspool,
        ):
            # bias8[p, mo] = sum_l bias[l, mo*128+p] / NCORES  (SWDGE: keep the
            # SP queue free for the first acts chunk)
            bias_t = cpool.tile([P, MD, n_layers], mybir.dt.float32)
            nc.gpsimd.dma_start(bias_t[:], b_v)
            bias8 = cpool.tile([P, MD], mybir.dt.float32)
            nc.vector.reduce_sum(bias8[:], bias_t[:], axis=mybir.AxisListType.X)
            nc.vector.tensor_scalar_mul(bias8[:], bias8[:], 1.0 / NCORES)

            def evac(blk, m, ps_m):
                """PSUM -> SBUF (+bias/8) -> partial DRAM, alternating engines."""
                ob = opool.tile([P, BN], mybir.dt.bfloat16, tag="o", name=f"ob{blk}_{m}")
                if m % 2 == 0:
                    nc.vector.tensor_scalar_add(ob[:], ps_m[:], bias8[:, m : m + 1])
                else:
                    nc.scalar.add(ob[:], ps_m[:], bias8[:, m : m + 1])
                nc.scalar.dma_start(partials[blk][ts(m, P), :], ob[:])

            w_tiles = []
            for blk in range(NBLK):
                b0 = blk * BN
                ps = [pspool.tile([P, BN], mybir.dt.float32, tag="ps", name=f"ps{blk}_{m}") for m in range(MD)]
                for c in range(NCH):
                    a_c = apool.tile([P, ck, BN], mybir.dt.bfloat16, tag="a")
                    if blk == 0:
                        w_c = wpool.tile([P, ck, D], mybir.dt.bfloat16, tag="w")
                        w_tiles.append(w_c)
                        if c == 0:
                            # per-k-tile DMAs so the first matmul waits on
                            # ~1/6 of the chunk, not all of it
                            for k in range(ck):
                                nc.sync.dma_start(a_c[:, k], a_v[:, k, b0 : b0 + BN])
                                nc.sync.dma_start(w_c[:, k], w_v[:, k, :])
                        else:
                            # same SP queue as acts: FIFO keeps the a/W streams
                            # interleaved per chunk (a separate queue lets all
                            # 12 ungated W DMAs flood the DMA engines and
                            # starve the compute-gated acts stream)
                            nc.sync.dma_start(a_c[:], a_v[:, c * ck : (c + 1) * ck, b0 : b0 + BN])
                            nc.sync.dma_start(w_c[:], w_v[:, c * ck : (c + 1) * ck, :])
                    else:
                        nc.sync.dma_start(a_c[:], a_v[:, c * ck : (c + 1) * ck, b0 : b0 + BN])
                        w_c = w_tiles[c]
                    if c < NCH - 1:
                        for k in range(ck):
                            for m in range(MD):
                                nc.tensor.matmul(
                                    ps[m][:],
                                    w_c[:, k, ts(m, P)],
                                    a_c[:, k],
                                    start=(c == 0 and k == 0),
                                    stop=False,
                                )
                    else:
                        # last chunk m-outer: each subtile's accumulation
                        # finishes early, its evacuation overlaps the rest
                        for m in range(MD):
                            for k in range(ck):
                                nc.tensor.matmul(
                                    ps[m][:],
                                    w_c[:, k, ts(m, P)],
                                    a_c[:, k],
                                    start=False,
                                    stop=(k == ck - 1),
                                )
                            evac(blk, m, ps[m])

                nc.gpsimd.collective_compute(
                    "ReduceScatter",
                    mybir.AluOpType.add,
                    replica_groups=[list(range(NCORES))],
                    ins=[partials[blk][:, :].opt()],
                    outs=[reduceds[blk][:, :].opt()],
                )
                # Pool-engine DMA converts the bf16 reduced shard to fp32 y
                nc.gpsimd.dma_start(y_ext[blk], reduceds[blk][:, :])
    nc.compile()
    return nc


def _get_nc(n_layers: int):
    if n_layers not in _nc_cache:
        _nc_cache[n_layers] = _build(n_layers)
    return _nc_cache[n_layers]


def kernel(acts: np.ndarray, W: np.ndarray, bias: np.ndarray, layer_idx) -> np.ndarray:
    global last_result
    n = int(layer_idx) + 1
    bf16 = ml_dtypes.bfloat16
    acts16 = np.asarray(acts, dtype=np.float32)[:n].astype(bf16)  # [n, B, F]
    W16 = np.asarray(W, dtype=np.float32)[:n].astype(bf16)        # [n, F, D]
    bias = np.asarray(bias, dtype=np.float32)[:n]                 # [n, D]

    nc = _get_nc(n)

    bias_t = np.ascontiguousarray(bias.T)  # [D, n], same on every core
    in_maps = []
    for r in range(NCORES):
        f0 = r * F_LOC
        # [n, B, F_LOC] -> [n, F_LOC, B] -> [K_loc, B]
        a_t = np.ascontiguousarray(
            acts16[:, :, f0 : f0 + F_LOC].transpose(0, 2, 1)
        ).reshape(n * F_LOC, B)
        w_r = np.ascontiguousarray(W16[:, f0 : f0 + F_LOC, :]).reshape(n * F_LOC, D)
        in_maps.append({"a_t": a_t, "w": w_r, "bias_t": bias_t})

    last_result = run_bass_kernel_spmd(nc, in_maps, core_ids=list(range(NCORES)))
    out = np.empty((D, B), dtype=np.float32)
    for r in range(NCORES):
        y_r = np.asarray(last_result.results[r]["y"], dtype=np.float32)  # [NBLK, DR, BN]
        for blk in range(NBLK):
            out[r * DR : (r + 1) * DR, blk * BN : (blk + 1) * BN] = y_r[blk]
    return np.ascontiguousarray(out.T)  # [B, D] float32
